# revision 1
# baseline (speedup 1.0000x reference)
"""Collective variant: K/V projection split across core pairs + pair AllGather.

Core c = (batch c//2, stripe h = c%2). Each core projects K^T and V only for
its own key half (s in [h*1024, (h+1)*1024)), then the pair exchanges halves
via two AllGathers (one per 512-key own-block) so attention can start as soon
as the first halves have been gathered.

Gathered DRAM layout (per 512-key global block b, r = b//2 = producing rank,
sub = b%2 selects which of the two collectives):
  cc = ccA if b%2==0 else ccB; base = r*2048
  KT tile k:  cc[base + k*128 : +128, :]                       [128, 512]
  V tile st:  cc[base + 1024 + st*256 : +256, :] as [128,1024] (row-pair fold)
"""

import numpy as np

B, S, E, KD = 4, 2048, 1024, 1024
NCORES = 8
P = 128
ET = E // P
KT = KD // P
NQT = 8
NBLK = 4
NEG = -30000.0
SCALE = 1.0 / float(np.sqrt(KD))

_prog_cache = {}


def _n_blocks(t):
    return (t + 2) // 2


def _build_body(ctx, tc, ap):
    from concourse import mybir
    from concourse.masks import make_identity

    nc = tc.nc
    f32 = mybir.dt.float32
    f32r = mybir.dt.float32r
    Exp = mybir.ActivationFunctionType.Exp
    X = mybir.AxisListType.X

    xTq_t = ap["xTq"].rearrange("(t p) q -> t p q", p=P)    # [8, 128, 1024]
    xTp_t = ap["xTp"].rearrange("(t p) s -> t p s", p=P)    # [8, 128, 1024]
    wqT_t = ap["wqT"].rearrange("(t p) k -> t p k", p=P)
    wkT_t = ap["wkT"].rearrange("(t p) k -> t p k", p=P)
    wvT_t = ap["wvT"].rearrange("(t p) f -> t p f", p=P)
    out_t = ap["out"].rearrange("(t p) f -> t p f", p=P)

    # ---- persistent tiles
    qt_pool = ctx.enter_context(tc.tile_pool(name="qt", bufs=1))
    QT = [qt_pool.tile([P, 1024], f32r, name=f"qt{k}", tag=f"qt{k}") for k in range(KT)]
    acc_pool = ctx.enter_context(tc.tile_pool(name="acc", bufs=1))
    OACC = [acc_pool.tile([P, E], f32, name=f"oacc{t}", tag=f"oacc{t}") for t in range(NQT)]
    RS = [acc_pool.tile([P, NBLK], f32, name=f"rs{t}", tag=f"rs{t}") for t in range(NQT)]
    const_pool = ctx.enter_context(tc.tile_pool(name="const", bufs=1))
    fin_pool = ctx.enter_context(tc.tile_pool(name="fin", bufs=4))

    # ---- DRAM tiles for the pair exchange
    dram = ctx.enter_context(tc.tile_pool(name="dram", bufs=1, space="DRAM"))
    ccin = [dram.tile([2048, 512], f32r, name=f"ccin{i}", tag=f"ccin{i}") for i in range(2)]
    ccout = [dram.tile([4096, 512], f32r, name=f"ccout{i}", tag=f"ccout{i}") for i in range(2)]

    # ---- PSUM: pp (projection evict) lives only through the projection
    # phases; its banks are then handed to the attention pools (vp bufs=2).
    pp_ctx = tc.tile_pool(name="pp", bufs=4, space="PSUM")
    pp = pp_ctx.__enter__()

    # ---- Phase A: own-half K/V projection + pair exchange.
    # Emitted FIRST so the K/V inputs arrive first and the collectives launch
    # as early as possible; the QT projection then runs underneath the
    # collective latency instead of in front of it.
    with tc.tile_pool(name="wkp", bufs=1) as wk_pool, \
         tc.tile_pool(name="wvp", bufs=1) as wv_pool, \
         tc.tile_pool(name="xpp", bufs=1) as xp_pool, \
         tc.tile_pool(name="stg", bufs=2) as stg_pool:
        wk = [wk_pool.tile([P, KD], f32r, name=f"wk{e}", tag=f"wk{e}") for e in range(ET)]
        xp = [xp_pool.tile([P, 1024], f32r, name=f"xp{e}", tag=f"xp{e}") for e in range(ET)]
        wv = [wv_pool.tile([P, E], f32r, name=f"wv{e}", tag=f"wv{e}") for e in range(ET)]
        # arrival order tuned to keep PE continuously fed:
        # [xp0+wk] -> xp1 -> wv -> (wq, xq emitted in phase B)
        for e in range(ET):
            nc.sync.dma_start(out=xp[e], in_=xTp_t[e])
            nc.sync.dma_start(out=wk[e], in_=wkT_t[e])
        for e in range(ET):
            nc.sync.dma_start(out=wv[e], in_=wvT_t[e])

        for ob in range(2):
            for k in range(KT):
                ps = pp.tile([P, 512], f32, name="ps_kt", tag="pp")
                for e in range(ET):
                    nc.tensor.matmul(ps, wk[e][:, k * P:(k + 1) * P],
                                     xp[e][:, ob * 512:(ob + 1) * 512],
                                     start=(e == 0), stop=(e == ET - 1))
                ko = stg_pool.tile([P, 512], f32r, name="ko", tag="ko", bufs=3)
                nc.vector.tensor_copy(ko, ps)
                nc.gpsimd.dma_start(out=ccin[ob][k * P:(k + 1) * P, :], in_=ko)
            # V_own[ob]: [512, 1024] -> rows 1024: as [1024, 512] row-pair fold
            for st in range(4):
                vo = stg_pool.tile([P, E], f32r, name="vo", tag="vo", bufs=3)
                for fb in range(2):
                    ps = pp.tile([P, 512], f32, name="ps_v", tag="pp")
                    for e in range(ET):
                        nc.tensor.matmul(
                            ps, xp[e][:, ob * 512 + st * P: ob * 512 + (st + 1) * P],
                            wv[e][:, fb * 512:(fb + 1) * 512],
                            start=(e == 0), stop=(e == ET - 1))
                    if fb == 0:
                        nc.scalar.copy(vo[:, fb * 512:(fb + 1) * 512], ps)
                    else:
                        nc.vector.tensor_copy(vo[:, fb * 512:(fb + 1) * 512], ps)
                vdst = ccin[ob][1024 + st * 256: 1024 + (st + 1) * 256, :]
                nc.gpsimd.dma_start(
                    out=vdst.rearrange("(s a) c -> s (a c)", a=2), in_=vo)
            nc.gpsimd.collective_compute(
                "AllGather", mybir.AluOpType.bypass,
                replica_groups=[[0, 1], [2, 3], [4, 5], [6, 7]],
                ins=[ccin[ob].opt()], outs=[ccout[ob].opt()],
            )

    # ---- Phase B: QT[k, q] projection (runs while the collectives fly)
    with tc.tile_pool(name="wqp", bufs=1) as wq_pool, \
         tc.tile_pool(name="xqp", bufs=1) as xq_pool:
        wq = [wq_pool.tile([P, KD], f32r, name=f"wq{e}", tag=f"wq{e}") for e in range(ET)]
        xq = [xq_pool.tile([P, 1024], f32r, name=f"xq{e}", tag=f"xq{e}") for e in range(ET)]
        for e in range(ET):
            nc.sync.dma_start(out=wq[e], in_=wqT_t[e])
            nc.sync.dma_start(out=xq[e], in_=xTq_t[e])
        for qb in range(2):
            for k in range(KT):
                ps = pp.tile([P, 512], f32, name="ps_qt", tag="pp")
                for e in range(ET):
                    nc.tensor.matmul(
                        ps, wq[e][:, k * P:(k + 1) * P],
                        xq[e][:, qb * 512:(qb + 1) * 512],
                        start=(e == 0), stop=(e == ET - 1))
                if k % 2 == 0:
                    nc.vector.tensor_copy(QT[k][:, qb * 512:(qb + 1) * 512], ps)
                else:
                    nc.scalar.copy(QT[k][:, qb * 512:(qb + 1) * 512], ps)

    # ---- Phase C: attention over global blocks
    pp_ctx.__exit__(None, None, None)
    cm = const_pool.tile([P, 256], f32, name="cm")
    nc.sync.dma_start(out=cm, in_=ap["cmask"])
    ident_f32 = const_pool.tile([P, P], f32, name="ident_f32")
    make_identity(nc, ident_f32)
    ident = const_pool.tile([P, P], f32r, name="ident")
    nc.vector.tensor_copy(ident, ident_f32)
    sp = ctx.enter_context(tc.tile_pool(name="sp", bufs=2, space="PSUM"))
    tp = ctx.enter_context(tc.tile_pool(name="tp", bufs=2, space="PSUM"))
    vp = ctx.enter_context(tc.tile_pool(name="vp", bufs=2, space="PSUM"))
    kt_pool = ctx.enter_context(tc.tile_pool(name="ktp", bufs=2))
    vb_pool = ctx.enter_context(tc.tile_pool(name="vbp", bufs=2))
    p_pool = ctx.enter_context(tc.tile_pool(name="ppb", bufs=4))
    pt_pool = ctx.enter_context(tc.tile_pool(name="ptp", bufs=6))

    ORDER = (0, 2, 1, 3)  # blocks 0,2 come from CC1 — start before CC2 lands
    last_visit = {t: [b for b in ORDER if t >= 2 * b][-1] for t in range(NQT)}

    def emit_pv(pend):
        # deferred transpose/copy/PV/accumulate for one (blk, t) work item;
        # runs one position behind the scores stream so the PE->DVE->PE
        # transpose-copy chain and exp latency hide behind matmul work.
        pb, w, blk, t, vbt = pend
        nst = w // P
        tpss = []
        for st in range(nst):
            tps = tp.tile([P, P], f32r, name="tps", tag="tp")
            nc.tensor.transpose(tps, pb[:, st * P:(st + 1) * P], ident)
            tpss.append(tps)
            if st > 0:
                pts = pt_pool.tile([P, P], f32r, name="pts", tag=f"pt{st-1}")
                nc.vector.tensor_copy(pts, tpss[st - 1])
                tpss[st - 1] = pts
        pts = pt_pool.tile([P, P], f32r, name="pts", tag=f"pt{nst-1}")
        nc.vector.tensor_copy(pts, tpss[nst - 1])
        tpss[nst - 1] = pts
        vps = [vp.tile([P, 512], f32, name=f"vps{fb}", tag=f"vp{fb}") for fb in range(2)]
        for st in range(nst):
            for fb in range(2):
                nc.tensor.matmul(vps[fb], tpss[st],
                                 vbt[st][:, fb * 512:(fb + 1) * 512],
                                 start=(st == 0), stop=(st == nst - 1))
        for fb in range(2):
            dst = OACC[t][:, fb * 512:(fb + 1) * 512]
            if blk == 0:
                nc.vector.tensor_copy(dst, vps[fb])
            else:
                nc.vector.tensor_add(dst, dst, vps[fb])
        if blk == last_visit[t]:
            nb = _n_blocks(t)
            rsum = fin_pool.tile([P, 1], f32, name="rsum", tag="rsum")
            nc.vector.reduce_sum(rsum, RS[t][:, :nb], axis=X)
            rinv = fin_pool.tile([P, 1], f32, name="rinv", tag="rinv")
            nc.vector.reciprocal(rinv, rsum)
            nc.scalar.activation(OACC[t], OACC[t],
                                 mybir.ActivationFunctionType.Copy, scale=rinv)
            nc.sync.dma_start(out=out_t[t], in_=OACC[t])

    pending = None  # pipeline carries across block boundaries (vb bufs=2)
    for blk in ORDER:
        r, sub = blk // 2, blk % 2
        cc = ccout[sub]
        base = r * 2048
        ktb = [kt_pool.tile([P, 512], f32r, name=f"ktb{k}", tag=f"ktb{k}") for k in range(KT)]
        for k in range(KT):
            nc.sync.dma_start(out=ktb[k], in_=cc[base + k * P: base + (k + 1) * P, :])
        vbt = [vb_pool.tile([P, E], f32r, name=f"vb{st}", tag=f"vb{st}") for st in range(4)]
        for st in range(4):
            vsrc = cc[base + 1024 + st * 256: base + 1024 + (st + 1) * 256, :]
            nc.sync.dma_start(out=vbt[st], in_=vsrc.rearrange("(s a) c -> s (a c)", a=2))

        for t in range(2 * blk, NQT):
            w = min(512, 256 * (t + 1) - 512 * blk)
            is_diag = (blk == _n_blocks(t) - 1)
            sps = sp.tile([P, 512], f32, name="sps", tag="sp")
            for k in range(KT):
                nc.tensor.matmul(sps[:, :w], QT[k][:, t * P:(t + 1) * P],
                                 ktb[k][:, :w], start=(k == 0), stop=(k == KT - 1))
            if is_diag:
                nc.vector.tensor_add(sps[:, w - 256:w], sps[:, w - 256:w], cm)
            pb = p_pool.tile([P, 512], f32r, name="pb", tag="pb")
            nc.scalar.activation(pb[:, :w], sps[:, :w], Exp, scale=SCALE,
                                 accum_out=RS[t][:, blk:blk + 1])
            if pending is not None:
                emit_pv(pending)
            pending = (pb, w, blk, t, vbt)
    emit_pv(pending)


def build_program():
    if "nc" in _prog_cache:
        return _prog_cache["nc"]
    from contextlib import ExitStack
    from concourse import bacc, mybir
    import concourse.tile as tile

    nc = bacc.Bacc("TRN2", target_bir_lowering=False, debug=False,
                   num_devices=NCORES)
    f32 = mybir.dt.float32
    f32r = mybir.dt.float32r
    ap = {
        "xTq": nc.dram_tensor("xTq", [E, 1024], f32r, kind="ExternalInput").ap(),
        "xTp": nc.dram_tensor("xTp", [E, 1024], f32r, kind="ExternalInput").ap(),
        "wqT": nc.dram_tensor("wqT", [E, KD], f32r, kind="ExternalInput").ap(),
        "wkT": nc.dram_tensor("wkT", [E, KD], f32r, kind="ExternalInput").ap(),
        "wvT": nc.dram_tensor("wvT", [E, E], f32r, kind="ExternalInput").ap(),
        "cmask": nc.dram_tensor("cmask", [P, 256], f32, kind="ExternalInput").ap(),
        "out": nc.dram_tensor("out", [1024, E], f32, kind="ExternalOutput").ap(),
    }
    with tile.TileContext(nc) as tc:
        with ExitStack() as ctx:
            _build_body(ctx, tc, ap)
    nc.compile()
    _prog_cache["nc"] = nc
    return nc


def make_in_maps(x, W_q, W_k, W_v):
    x = np.ascontiguousarray(np.asarray(x, np.float32))
    wqT = np.ascontiguousarray(np.asarray(W_q, np.float32).T)
    wkT = np.ascontiguousarray(np.asarray(W_k, np.float32).T)
    wvT = np.ascontiguousarray(np.asarray(W_v, np.float32).T)
    i = np.arange(P)[:, None]
    j = np.arange(256)[None, :]
    cmasks = [np.where(j <= i + 128, 0.0, NEG).astype(np.float32),
              np.where(j <= i, 0.0, NEG).astype(np.float32)]
    in_maps = []
    for c in range(NCORES):
        b, h = c // 2, c % 2
        xT = np.ascontiguousarray(x[b].T)
        qtiles = [2 * t + (1 - h) for t in range(NQT)]
        qcols = np.concatenate([np.arange(g * P, (g + 1) * P) for g in qtiles])
        xTq = np.ascontiguousarray(xT[:, qcols])
        xTp = np.ascontiguousarray(xT[:, h * 1024:(h + 1) * 1024])
        in_maps.append({
            "xTq": xTq, "xTp": xTp, "wqT": wqT, "wkT": wkT, "wvT": wvT,
            "cmask": cmasks[h],
        })
    return in_maps


def assemble(results):
    out = np.zeros((B, S, E), np.float32)
    for c in range(NCORES):
        b, h = c // 2, c % 2
        co = results[c]["out"]
        for t in range(NQT):
            g = 2 * t + (1 - h)
            out[b, g * P:(g + 1) * P, :] = co[t * P:(t + 1) * P]
    return out


def kernel(x, W_q, W_k, W_v):
    from concourse.bass_utils import run_bass_kernel_spmd
    nc = build_program()
    in_maps = make_in_maps(x, W_q, W_k, W_v)
    res = run_bass_kernel_spmd(nc, in_maps, core_ids=list(range(NCORES)))
    return assemble(res.results)



# revision 7
# speedup vs baseline: 3.9770x; 3.9770x over previous
"""Collective-free causal attention: scores = x(Wq^T Wk)x^T, out = (P x)Wv^T.

Core c = (batch c//2, query-stripe h = c%2); stripe h owns interleaved
128-row query tiles g = 2t + (1-h), t in 0..8, which balances the causal
triangle across the pair without any cross-core communication.

Device math (all matmul inputs bf16, accumulation f32 in PSUM):
  M    = Wq^T Wk * scale            (host, weight-only preprocessing)
  A^T  = M^T x_q^T                  [e', q]    phase A
  S^T  = x^T(stripes) . A^T         [s, q]     per (t, s-tile), N=128
  P^T  = exp(S^T) (* tri-mask on the 1-2 diagonal tiles, data-driven)
  r    = P^T^T @ ones               rowsums via PE, PSUM-accumulated
  Z^T  = x . P^T                    [e, q]     PSUM-accumulated over s
  out  = (Z^T^T @ Wv^T) * (1/r)     [q, f]
The [s,q] layout means exp output feeds the PV/rowsum matmuls directly as
the stationary operand - no PE transposes anywhere.
"""

import numpy as np

B, S, E, KD = 4, 2048, 1024, 1024
NCORES = 8
P = 128
NQT = 8          # own query tiles per core
NST = 16         # 128-row key tiles per batch
SCALE = 1.0 / float(np.sqrt(KD))

PIPE = 2         # score->exp->PV software pipeline depth (in st-steps)
OPDELAY = 2      # st-steps into tile t+1 before emitting out-proj of t

_prog_cache = {}


def _build_body(ctx, tc, ap):
    from concourse import mybir

    nc = tc.nc
    f32 = mybir.dt.float32
    bf16 = mybir.dt.bfloat16
    Exp = mybir.ActivationFunctionType.Exp
    Copy = mybir.ActivationFunctionType.Copy

    # ---- persistent SBUF inputs (packed [128, k*cols] fold layouts)
    wp = ctx.enter_context(tc.tile_pool(name="wp", bufs=1))
    m_sb = wp.tile([P, 8 * 1024], bf16, name="m_sb")
    xtq_sb = wp.tile([P, 8 * 1024], bf16, name="xtq_sb")
    xts_sb = wp.tile([P, 8 * 2048], bf16, name="xts_sb")
    xn_sb = wp.tile([P, 16 * 1024], bf16, name="xn_sb")
    wvt_sb = wp.tile([P, 8 * 1024], bf16, name="wvt_sb")
    at_sb = wp.tile([P, 8 * 1024], bf16, name="at_sb")
    maskp = wp.tile([P, P], bf16, name="maskp")
    maskl = wp.tile([P, P], bf16, name="maskl")
    ones = wp.tile([P, 1], bf16, name="ones")

    nc.vector.memset(ones, 1.0)

    # ---- input DMAs, ordered by first use
    for et in range(8):
        nc.sync.dma_start(out=m_sb[:, et * 1024:(et + 1) * 1024],
                          in_=ap["m"][:, et * 1024:(et + 1) * 1024])
        nc.sync.dma_start(out=xtq_sb[:, et * 1024:(et + 1) * 1024],
                          in_=ap["xtq"][:, et * 1024:(et + 1) * 1024])
    nc.sync.dma_start(out=maskp, in_=ap["maskp"])
    nc.sync.dma_start(out=maskl, in_=ap["maskl"])
    for c in range(16):
        nc.sync.dma_start(out=xts_sb[:, c * 1024:(c + 1) * 1024],
                          in_=ap["xts"][:, c * 1024:(c + 1) * 1024])
    for c in range(2):
        nc.sync.dma_start(out=xn_sb[:, c * 1024:(c + 1) * 1024],
                          in_=ap["xn"][:, c * 1024:(c + 1) * 1024])
    for ez in range(8):
        nc.sync.dma_start(out=wvt_sb[:, ez * 1024:(ez + 1) * 1024],
                          in_=ap["wvt"][:, ez * 1024:(ez + 1) * 1024])
    for c in range(2, 16):
        nc.sync.dma_start(out=xn_sb[:, c * 1024:(c + 1) * 1024],
                          in_=ap["xn"][:, c * 1024:(c + 1) * 1024])

    # ---- phase A: A^T[e',q] = sum_e M[e,e'] xTq[e,q]
    with tc.tile_pool(name="pa", bufs=3, space="PSUM") as pa:
        for ep in range(8):
            for hf in range(2):
                ps = pa.tile([P, 512], f32, name="pa", tag="pa")
                for et in range(8):
                    nc.tensor.matmul(
                        ps,
                        m_sb[:, et * 1024 + ep * P: et * 1024 + (ep + 1) * P],
                        xtq_sb[:, et * 1024 + hf * 512: et * 1024 + (hf + 1) * 512],
                        start=(et == 0), stop=(et == 7))
                dst = at_sb[:, ep * 1024 + hf * 512: ep * 1024 + (hf + 1) * 512]
                if hf == 0:
                    nc.scalar.copy(dst, ps)
                else:
                    nc.vector.tensor_copy(dst, ps)

    # ---- phase B pools (PSUM banks: sp 2 + zt 4 + rs 1 + op 1 = 8)
    sp = ctx.enter_context(tc.tile_pool(name="sp", bufs=2, space="PSUM"))
    ztp = ctx.enter_context(tc.tile_pool(name="ztp", bufs=2, space="PSUM"))
    rsp = ctx.enter_context(tc.tile_pool(name="rsp", bufs=1, space="PSUM"))
    opp = ctx.enter_context(tc.tile_pool(name="opp", bufs=1, space="PSUM"))
    ptp = ctx.enter_context(tc.tile_pool(name="ptp", bufs=PIPE + 3))
    ztsb = ctx.enter_context(tc.tile_pool(name="ztsb", bufs=2))
    rvp = ctx.enter_context(tc.tile_pool(name="rvp", bufs=2))
    osp = ctx.enter_context(tc.tile_pool(name="osp", bufs=3))

    out_t = ap["out"].rearrange("(t p) f -> t p f", p=P)

    steps = [(t, st) for t in range(NQT) for st in range(2 * t + 2)]
    rs = rsp.tile([P, 1], f32, name="rs")  # one bank, reused every t
    state = {}   # t -> zt tiles ([128,512] x2, 4 e-slices each)
    zts_of = {}  # t -> evicted SBUF zt tiles
    rinv_of = {}
    spcur = []   # rolling [128,512] score tile, 4 st-slices

    def emit_scores(i, t, st):
        g = 2 * t + 1
        if i % 4 == 0:
            spcur.append(sp.tile([P, 512], f32, name="sps", tag="sp"))
            if len(spcur) > 1:
                spcur.pop(0)
        ps = spcur[-1][:, (i % 4) * P:(i % 4 + 1) * P]
        for ep in range(8):
            nc.tensor.matmul(
                ps,
                xts_sb[:, ep * 2048 + st * P: ep * 2048 + (st + 1) * P],
                at_sb[:, ep * 1024 + t * P: ep * 1024 + (t + 1) * P],
                start=(ep == 0), stop=(ep == 7))
        pt = ptp.tile([P, P], bf16, name="pt", tag="pt")
        nc.scalar.activation(pt, ps, Exp)
        if st == g - 1:
            nc.vector.tensor_mul(pt, pt, maskp)
        elif st == g:
            nc.vector.tensor_mul(pt, pt, maskl)
        return pt

    def emit_op(t, hf):
        zs = zts_of[t]
        rv = rinv_of[t]
        po = opp.tile([P, 512], f32, name="po", tag="op")
        for ez in range(8):
            nc.tensor.matmul(
                po, zs[ez // 4][:, (ez % 4) * P:(ez % 4 + 1) * P],
                wvt_sb[:, ez * 1024 + hf * 512: ez * 1024 + (hf + 1) * 512],
                start=(ez == 0), stop=(ez == 7))
        ob = osp.tile([P, 512], f32, name="ob", tag="ob")
        nc.scalar.activation(ob, po, Copy, scale=rv)
        nc.sync.dma_start(out=out_t[t][:, hf * 512:(hf + 1) * 512], in_=ob)

    def emit_rz(t, st, pt):
        g = 2 * t + 1
        if st == 0:
            state[t] = [ztp.tile([P, 512], f32, name=f"zt{j}", tag=f"zt{j}")
                        for j in range(2)]
        zt = state[t]
        nc.tensor.matmul(rs, pt, ones, start=(st == 0), stop=(st == g))
        for ez in range(8):
            # one accumulation group per zt tile: start/stop only on that
            # tile's first/last matmul of the whole st loop (2KB zero region)
            nc.tensor.matmul(
                zt[ez // 4][:, (ez % 4) * P:(ez % 4 + 1) * P],
                xn_sb[:, st * 1024 + ez * P: st * 1024 + (ez + 1) * P],
                pt,
                start=(st == 0 and ez % 4 == 0),
                stop=(st == g and ez % 4 == 3))
        if st == g:
            zs = []
            for j in range(2):
                z = ztsb.tile([P, 512], bf16, name=f"zs{j}", tag=f"zs{j}")
                if j == 0:
                    nc.scalar.copy(z, zt[j])
                else:
                    nc.vector.tensor_copy(z, zt[j])
                zs.append(z)
            zts_of[t] = zs
            rv = rvp.tile([P, 1], f32, name="rv", tag="rv")
            nc.vector.reciprocal(rv, rs)
            rinv_of[t] = rv
            del state[t]
        elif t > 0 and st == 0:
            emit_op(t - 1, 0)
        elif t > 0 and st == 2:
            emit_op(t - 1, 1)

    pend = []
    for i in range(len(steps) + PIPE):
        if i < len(steps):
            t, st = steps[i]
            pend.append((t, st, emit_scores(i, t, st)))
        if i >= PIPE:
            t, st, pt = pend.pop(0)
            emit_rz(t, st, pt)
    emit_op(NQT - 1, 0)
    emit_op(NQT - 1, 1)


def build_program():
    if "nc" in _prog_cache:
        return _prog_cache["nc"]
    from contextlib import ExitStack
    from concourse import bacc, mybir
    import concourse.tile as tile

    nc = bacc.Bacc("TRN2", target_bir_lowering=False, debug=False,
                   num_devices=NCORES)
    f32 = mybir.dt.float32
    bf16 = mybir.dt.bfloat16
    ap = {
        "m": nc.dram_tensor("m", [P, 8 * 1024], bf16, kind="ExternalInput").ap(),
        "xtq": nc.dram_tensor("xtq", [P, 8 * 1024], bf16, kind="ExternalInput").ap(),
        "xts": nc.dram_tensor("xts", [P, 16 * 1024], bf16, kind="ExternalInput").ap(),
        "xn": nc.dram_tensor("xn", [P, 16 * 1024], bf16, kind="ExternalInput").ap(),
        "wvt": nc.dram_tensor("wvt", [P, 8 * 1024], bf16, kind="ExternalInput").ap(),
        "maskp": nc.dram_tensor("maskp", [P, P], bf16, kind="ExternalInput").ap(),
        "maskl": nc.dram_tensor("maskl", [P, P], bf16, kind="ExternalInput").ap(),
        "out": nc.dram_tensor("out", [1024, E], f32, kind="ExternalOutput").ap(),
    }
    with tile.TileContext(nc) as tc:
        with ExitStack() as ctx:
            _build_body(ctx, tc, ap)
    nc.compile()
    _prog_cache["nc"] = nc
    return nc


def _fold(a, nt, cols):
    # [nt*128, cols] -> [128, nt*cols] with block j at cols [j*cols:(j+1)*cols]
    return np.ascontiguousarray(
        a.reshape(nt, P, cols).transpose(1, 0, 2).reshape(P, nt * cols))


def make_in_maps(x, W_q, W_k, W_v):
    import ml_dtypes
    bf = ml_dtypes.bfloat16
    x = np.asarray(x, np.float32)
    W_q = np.asarray(W_q, np.float32)
    W_k = np.asarray(W_k, np.float32)
    W_v = np.asarray(W_v, np.float32)

    M = (W_q.T @ W_k) * SCALE                      # [e, e'], scale folded
    m_p = _fold(M, 8, 1024).astype(bf)
    wvt_p = _fold(np.ascontiguousarray(W_v.T), 8, 1024).astype(bf)

    i = np.arange(P)[:, None]
    j = np.arange(P)[None, :]
    tri = (i <= j).astype(np.float32)              # allow s_local <= q_local
    masks = [(np.ones((P, P), np.float32), tri),   # h=0: odd tiles, diag last
             (tri, np.zeros((P, P), np.float32))]  # h=1: even tiles

    in_maps = []
    for c in range(NCORES):
        b, h = c // 2, c % 2
        xb = x[b]                                  # [2048, 1024]
        xT = np.ascontiguousarray(xb.T)            # [1024, 2048]
        qcols = np.concatenate(
            [np.arange((2 * t + 1 - h) * P, (2 * t + 2 - h) * P)
             for t in range(NQT)])
        xq = np.ascontiguousarray(xb[qcols].T)     # [1024 e, 1024 q]
        mp, ml = masks[h]
        in_maps.append({
            "m": m_p,
            "xtq": _fold(xq, 8, 1024).astype(bf),
            "xts": _fold(xT, 8, 2048).astype(bf),
            "xn": _fold(xb, 16, 1024).astype(bf),
            "wvt": wvt_p,
            "maskp": mp.astype(bf),
            "maskl": ml.astype(bf),
        })
    return in_maps


def assemble(results):
    out = np.zeros((B, S, E), np.float32)
    for c in range(NCORES):
        b, h = c // 2, c % 2
        co = results[c]["out"]
        for t in range(NQT):
            g = 2 * t + (1 - h)
            out[b, g * P:(g + 1) * P, :] = co[t * P:(t + 1) * P]
    return out


def kernel(x, W_q, W_k, W_v):
    from concourse.bass_utils import run_bass_kernel_spmd
    nc = build_program()
    in_maps = make_in_maps(x, W_q, W_k, W_v)
    res = run_bass_kernel_spmd(nc, in_maps, core_ids=list(range(NCORES)))
    return assemble(res.results)


# revision 10
# speedup vs baseline: 4.2350x; 1.0649x over previous
"""Collective-free causal attention: scores = x(Wq^T Wk)x^T, out = (P x)Wv^T.

Core c = (batch c//2, query-stripe h = c%2); stripe h owns interleaved
128-row query tiles g = 2t + (1-h), t in 0..8, which balances the causal
triangle across the pair without any cross-core communication.

Device math (all matmul inputs bf16, accumulation f32 in PSUM):
  M    = Wq^T Wk * scale            (host, weight-only preprocessing)
  A^T  = M^T x_q^T                  [e', q]    phase A
  S^T  = x^T(stripes) . A^T         [s, q]     per (t, s-tile), N=128
  P^T  = exp(S^T) (* tri-mask on the 1-2 diagonal tiles, data-driven)
  r    = P^T^T @ ones               rowsums via PE, PSUM-accumulated
  Z^T  = x . P^T                    [e, q]     PSUM-accumulated over s
  out  = (Z^T^T @ Wv^T) * (1/r)     [q, f]
The [s,q] layout means exp output feeds the PV/rowsum matmuls directly as
the stationary operand - no PE transposes anywhere.
"""

import numpy as np

B, S, E, KD = 4, 2048, 1024, 1024
NCORES = 8
P = 128
NQT = 8          # own query tiles per core
NST = 16         # 128-row key tiles per batch
SCALE = 1.0 / float(np.sqrt(KD))

PIPE = 2         # score->exp->PV software pipeline depth (in st-steps)
OPDELAY = 2      # st-steps into tile t+1 before emitting out-proj of t

_prog_cache = {}


def _build_body(ctx, tc, ap):
    from concourse import mybir

    nc = tc.nc
    f32 = mybir.dt.float32
    bf16 = mybir.dt.bfloat16
    Exp = mybir.ActivationFunctionType.Exp
    Copy = mybir.ActivationFunctionType.Copy

    # ---- persistent SBUF inputs (packed [128, k*cols] fold layouts)
    wp = ctx.enter_context(tc.tile_pool(name="wp", bufs=1))
    m_sb = wp.tile([P, 8 * 1024], bf16, name="m_sb")
    xtq_sb = wp.tile([P, 8 * 1024], bf16, name="xtq_sb")
    xts_sb = wp.tile([P, 8 * 2048], bf16, name="xts_sb")
    xn_sb = wp.tile([P, 16 * 1024], bf16, name="xn_sb")
    wvt_sb = wp.tile([P, 8 * 1024], bf16, name="wvt_sb")
    at_sb = wp.tile([P, 8 * 1024], bf16, name="at_sb")
    maskp = wp.tile([P, P], bf16, name="maskp")
    maskl = wp.tile([P, P], bf16, name="maskl")
    ones = wp.tile([P, 1], bf16, name="ones")

    nc.vector.memset(ones, 1.0)

    # ---- input DMAs, ordered by first use
    for et in range(8):
        nc.sync.dma_start(out=m_sb[:, et * 1024:(et + 1) * 1024],
                          in_=ap["m"][:, et * 1024:(et + 1) * 1024])
        nc.sync.dma_start(out=xtq_sb[:, et * 1024:(et + 1) * 1024],
                          in_=ap["xtq"][:, et * 1024:(et + 1) * 1024])
    nc.sync.dma_start(out=maskp, in_=ap["maskp"])
    nc.sync.dma_start(out=maskl, in_=ap["maskl"])
    for c in range(16):
        nc.sync.dma_start(out=xts_sb[:, c * 1024:(c + 1) * 1024],
                          in_=ap["xts"][:, c * 1024:(c + 1) * 1024])
    for c in range(2):
        nc.sync.dma_start(out=xn_sb[:, c * 1024:(c + 1) * 1024],
                          in_=ap["xn"][:, c * 1024:(c + 1) * 1024])
    for ez in range(8):
        nc.sync.dma_start(out=wvt_sb[:, ez * 1024:(ez + 1) * 1024],
                          in_=ap["wvt"][:, ez * 1024:(ez + 1) * 1024])
    for c in range(2, 16):
        nc.sync.dma_start(out=xn_sb[:, c * 1024:(c + 1) * 1024],
                          in_=ap["xn"][:, c * 1024:(c + 1) * 1024])

    # ---- phase A: A^T[e',q] = sum_e M[e,e'] xTq[e,q]
    # et-major with 8 concurrent PSUM groups (one bank per e'-tile): the PE
    # consumes (m, xtq) DMA chunk pairs in arrival order instead of waiting
    # for the full 4MB before the first accumulation group can close.
    with tc.tile_pool(name="pa", bufs=1, space="PSUM") as pa:
        for hf in range(2):
            ps = [pa.tile([P, 512], f32, name=f"pa{ep}", tag=f"pa{ep}")
                  for ep in range(8)]
            for et in range(8):
                for ep in range(8):
                    nc.tensor.matmul(
                        ps[ep],
                        m_sb[:, et * 1024 + ep * P: et * 1024 + (ep + 1) * P],
                        xtq_sb[:, et * 1024 + hf * 512: et * 1024 + (hf + 1) * 512],
                        start=(et == 0), stop=(et == 7))
            for ep in range(8):
                dst = at_sb[:, ep * 1024 + hf * 512: ep * 1024 + (hf + 1) * 512]
                if ep % 2 == 0:
                    nc.scalar.copy(dst, ps[ep])
                else:
                    nc.vector.tensor_copy(dst, ps[ep])

    # ---- phase B pools (PSUM banks: sp 2 + zt 4 + rs 1 + op 1 = 8)
    sp = ctx.enter_context(tc.tile_pool(name="sp", bufs=1, space="PSUM"))
    ztp = ctx.enter_context(tc.tile_pool(name="ztp", bufs=2, space="PSUM"))
    rsp = ctx.enter_context(tc.tile_pool(name="rsp", bufs=1, space="PSUM"))
    opp = ctx.enter_context(tc.tile_pool(name="opp", bufs=1, space="PSUM"))
    ptp = ctx.enter_context(tc.tile_pool(name="ptp", bufs=PIPE + 3))
    ztsb = ctx.enter_context(tc.tile_pool(name="ztsb", bufs=2))
    rvp = ctx.enter_context(tc.tile_pool(name="rvp", bufs=2))
    osp = ctx.enter_context(tc.tile_pool(name="osp", bufs=3))

    out_t = ap["out"].rearrange("(t p) f -> t p f", p=P)

    steps = [(t, st) for t in range(NQT) for st in range(2 * t + 2)]
    rs = rsp.tile([P, 1], f32, name="rs")  # one bank, reused every t
    state = {}   # t -> zt tiles ([128,512] x2, 4 e-slices each)
    zts_of = {}  # t -> evicted SBUF zt tiles
    rinv_of = {}
    # Two score banks, alternating per step: a matmul group's start=True
    # write-locks its whole 2KB zero region, so consecutive steps must use
    # different banks or each step serializes on the previous step's exp read.
    spcur = {}   # parity -> rolling [128,512] tile, 4 st-slices

    def emit_scores(i, t, st):
        g = 2 * t + 1
        par, n = i % 2, i // 2
        if n % 4 == 0:
            spcur[par] = sp.tile([P, 512], f32, name=f"sps{par}",
                                 tag=f"sp{par}")
        ps = spcur[par][:, (n % 4) * P:(n % 4 + 1) * P]
        for ep in range(8):
            nc.tensor.matmul(
                ps,
                xts_sb[:, ep * 2048 + st * P: ep * 2048 + (st + 1) * P],
                at_sb[:, ep * 1024 + t * P: ep * 1024 + (t + 1) * P],
                start=(ep == 0), stop=(ep == 7))
        pt = ptp.tile([P, P], bf16, name="pt", tag="pt")
        nc.scalar.activation(pt, ps, Exp)
        if st == g - 1:
            nc.vector.tensor_mul(pt, pt, maskp)
        elif st == g:
            nc.vector.tensor_mul(pt, pt, maskl)
        return pt

    def emit_op(t, hf):
        zs = zts_of[t]
        rv = rinv_of[t]
        po = opp.tile([P, 512], f32, name="po", tag="op")
        for ez in range(8):
            nc.tensor.matmul(
                po, zs[ez // 4][:, (ez % 4) * P:(ez % 4 + 1) * P],
                wvt_sb[:, ez * 1024 + hf * 512: ez * 1024 + (hf + 1) * 512],
                start=(ez == 0), stop=(ez == 7))
        ob = osp.tile([P, 512], f32, name="ob", tag="ob")
        nc.scalar.activation(ob, po, Copy, scale=rv)
        nc.sync.dma_start(out=out_t[t][:, hf * 512:(hf + 1) * 512], in_=ob)

    def emit_rz(t, st, pt):
        g = 2 * t + 1
        if st == 0:
            state[t] = [ztp.tile([P, 512], f32, name=f"zt{j}", tag=f"zt{j}")
                        for j in range(2)]
        zt = state[t]
        nc.tensor.matmul(rs, pt, ones, start=(st == 0), stop=(st == g))
        for ez in range(8):
            # one accumulation group per zt tile: start/stop only on that
            # tile's first/last matmul of the whole st loop (2KB zero region)
            nc.tensor.matmul(
                zt[ez // 4][:, (ez % 4) * P:(ez % 4 + 1) * P],
                xn_sb[:, st * 1024 + ez * P: st * 1024 + (ez + 1) * P],
                pt,
                start=(st == 0 and ez % 4 == 0),
                stop=(st == g and ez % 4 == 3))
        if st == g:
            zs = []
            for j in range(2):
                z = ztsb.tile([P, 512], bf16, name=f"zs{j}", tag=f"zs{j}")
                if j == 0:
                    nc.scalar.copy(z, zt[j])
                else:
                    nc.vector.tensor_copy(z, zt[j])
                zs.append(z)
            zts_of[t] = zs
            rv = rvp.tile([P, 1], f32, name="rv", tag="rv")
            nc.vector.reciprocal(rv, rs)
            rinv_of[t] = rv
            del state[t]
        elif t > 0 and st == 0:
            emit_op(t - 1, 0)
        elif t > 0 and st == 2:
            emit_op(t - 1, 1)

    pend = []
    for i in range(len(steps) + PIPE):
        if i < len(steps):
            t, st = steps[i]
            pend.append((t, st, emit_scores(i, t, st)))
        if i >= PIPE:
            t, st, pt = pend.pop(0)
            emit_rz(t, st, pt)
    emit_op(NQT - 1, 0)
    emit_op(NQT - 1, 1)


def build_program():
    if "nc" in _prog_cache:
        return _prog_cache["nc"]
    from contextlib import ExitStack
    from concourse import bacc, mybir
    import concourse.tile as tile

    nc = bacc.Bacc("TRN2", target_bir_lowering=False, debug=False,
                   num_devices=NCORES)
    f32 = mybir.dt.float32
    bf16 = mybir.dt.bfloat16
    ap = {
        "m": nc.dram_tensor("m", [P, 8 * 1024], bf16, kind="ExternalInput").ap(),
        "xtq": nc.dram_tensor("xtq", [P, 8 * 1024], bf16, kind="ExternalInput").ap(),
        "xts": nc.dram_tensor("xts", [P, 16 * 1024], bf16, kind="ExternalInput").ap(),
        "xn": nc.dram_tensor("xn", [P, 16 * 1024], bf16, kind="ExternalInput").ap(),
        "wvt": nc.dram_tensor("wvt", [P, 8 * 1024], bf16, kind="ExternalInput").ap(),
        "maskp": nc.dram_tensor("maskp", [P, P], bf16, kind="ExternalInput").ap(),
        "maskl": nc.dram_tensor("maskl", [P, P], bf16, kind="ExternalInput").ap(),
        "out": nc.dram_tensor("out", [1024, E], f32, kind="ExternalOutput").ap(),
    }
    with tile.TileContext(nc) as tc:
        with ExitStack() as ctx:
            _build_body(ctx, tc, ap)
    nc.compile()
    _prog_cache["nc"] = nc
    return nc


def _fold(a, nt, cols):
    # [nt*128, cols] -> [128, nt*cols] with block j at cols [j*cols:(j+1)*cols]
    return np.ascontiguousarray(
        a.reshape(nt, P, cols).transpose(1, 0, 2).reshape(P, nt * cols))


def make_in_maps(x, W_q, W_k, W_v):
    import ml_dtypes
    bf = ml_dtypes.bfloat16
    x = np.asarray(x, np.float32)
    W_q = np.asarray(W_q, np.float32)
    W_k = np.asarray(W_k, np.float32)
    W_v = np.asarray(W_v, np.float32)

    M = (W_q.T @ W_k) * SCALE                      # [e, e'], scale folded
    m_p = _fold(M, 8, 1024).astype(bf)
    wvt_p = _fold(np.ascontiguousarray(W_v.T), 8, 1024).astype(bf)

    i = np.arange(P)[:, None]
    j = np.arange(P)[None, :]
    tri = (i <= j).astype(np.float32)              # allow s_local <= q_local
    masks = [(np.ones((P, P), np.float32), tri),   # h=0: odd tiles, diag last
             (tri, np.zeros((P, P), np.float32))]  # h=1: even tiles

    in_maps = []
    for c in range(NCORES):
        b, h = c // 2, c % 2
        xb = x[b]                                  # [2048, 1024]
        xT = np.ascontiguousarray(xb.T)            # [1024, 2048]
        qcols = np.concatenate(
            [np.arange((2 * t + 1 - h) * P, (2 * t + 2 - h) * P)
             for t in range(NQT)])
        xq = np.ascontiguousarray(xb[qcols].T)     # [1024 e, 1024 q]
        mp, ml = masks[h]
        in_maps.append({
            "m": m_p,
            "xtq": _fold(xq, 8, 1024).astype(bf),
            "xts": _fold(xT, 8, 2048).astype(bf),
            "xn": _fold(xb, 16, 1024).astype(bf),
            "wvt": wvt_p,
            "maskp": mp.astype(bf),
            "maskl": ml.astype(bf),
        })
    return in_maps


def assemble(results):
    out = np.zeros((B, S, E), np.float32)
    for c in range(NCORES):
        b, h = c // 2, c % 2
        co = results[c]["out"]
        for t in range(NQT):
            g = 2 * t + (1 - h)
            out[b, g * P:(g + 1) * P, :] = co[t * P:(t + 1) * P]
    return out


def kernel(x, W_q, W_k, W_v):
    from concourse.bass_utils import run_bass_kernel_spmd
    nc = build_program()
    in_maps = make_in_maps(x, W_q, W_k, W_v)
    res = run_bass_kernel_spmd(nc, in_maps, core_ids=list(range(NCORES)))
    return assemble(res.results)


# revision 16
# speedup vs baseline: 4.3582x; 1.0291x over previous
"""Collective-free causal attention: scores = x(Wq^T Wk)x^T, out = (P x)Wv^T.

Core c = (batch c//2, query-stripe h = c%2); stripe h owns interleaved
128-row query tiles g = 2t + (1-h), t in 0..8, which balances the causal
triangle across the pair without any cross-core communication.

Device math (all matmul inputs bf16, accumulation f32 in PSUM):
  M    = Wq^T Wk * scale            (host, weight-only preprocessing)
  A^T  = M^T x_q^T                  [e', q]    phase A
  S^T  = x^T(stripes) . A^T         [s, q]     per (t, s-tile), N=128
  P^T  = exp(S^T) (* tri-mask on the 1-2 diagonal tiles, data-driven)
  r    = P^T^T @ ones               rowsums via PE, PSUM-accumulated
  Z^T  = x . P^T                    [e, q]     PSUM-accumulated over s
  out  = (Z^T^T @ Wv^T) * (1/r)     [q, f]
The [s,q] layout means exp output feeds the PV/rowsum matmuls directly as
the stationary operand - no PE transposes anywhere.
"""

import numpy as np

B, S, E, KD = 4, 2048, 1024, 1024
NCORES = 8
P = 128
NQT = 8          # own query tiles per core
NST = 16         # 128-row key tiles per batch
SCALE = 1.0 / float(np.sqrt(KD))

PIPE = 3         # score->exp->PV software pipeline depth (in st-steps)

_prog_cache = {}


def _build_body(ctx, tc, ap):
    from concourse import mybir

    nc = tc.nc
    f32 = mybir.dt.float32
    bf16 = mybir.dt.bfloat16
    Exp = mybir.ActivationFunctionType.Exp
    Copy = mybir.ActivationFunctionType.Copy

    # ---- persistent SBUF inputs (packed [128, k*cols] fold layouts)
    wp = ctx.enter_context(tc.tile_pool(name="wp", bufs=1))
    m_sb = wp.tile([P, 8 * 1024], bf16, name="m_sb")
    xtq_sb = wp.tile([P, 8 * 1024], bf16, name="xtq_sb")
    xts_sb = wp.tile([P, 8 * 2048], bf16, name="xts_sb")
    xn_sb = wp.tile([P, 16 * 1024], bf16, name="xn_sb")
    wvt_sb = wp.tile([P, 8 * 1024], bf16, name="wvt_sb")
    at_sb = wp.tile([P, 8 * 1024], bf16, name="at_sb")
    maskp = wp.tile([P, P], bf16, name="maskp")
    maskl = wp.tile([P, P], bf16, name="maskl")
    ones = wp.tile([P, 1], bf16, name="ones")

    nc.vector.memset(ones, 1.0)

    # ---- input DMAs, ordered by first use (first chunk split finer so the
    # first phase-A matmuls start ~0.7us sooner)
    for c0, c1 in ((0, 512), (512, 1024)):
        nc.sync.dma_start(out=m_sb[:, c0:c1], in_=ap["m"][:, c0:c1])
        nc.sync.dma_start(out=xtq_sb[:, c0:c1], in_=ap["xtq"][:, c0:c1])
    for et in range(1, 8):
        nc.sync.dma_start(out=m_sb[:, et * 1024:(et + 1) * 1024],
                          in_=ap["m"][:, et * 1024:(et + 1) * 1024])
        nc.sync.dma_start(out=xtq_sb[:, et * 1024:(et + 1) * 1024],
                          in_=ap["xtq"][:, et * 1024:(et + 1) * 1024])
    nc.sync.dma_start(out=maskp, in_=ap["maskp"])
    nc.sync.dma_start(out=maskl, in_=ap["maskl"])
    for c in range(16):
        nc.sync.dma_start(out=xts_sb[:, c * 1024:(c + 1) * 1024],
                          in_=ap["xts"][:, c * 1024:(c + 1) * 1024])
    for c in range(2):
        nc.sync.dma_start(out=xn_sb[:, c * 1024:(c + 1) * 1024],
                          in_=ap["xn"][:, c * 1024:(c + 1) * 1024])
    for ez in range(8):
        nc.sync.dma_start(out=wvt_sb[:, ez * 1024:(ez + 1) * 1024],
                          in_=ap["wvt"][:, ez * 1024:(ez + 1) * 1024])
    for c in range(2, 16):
        nc.sync.dma_start(out=xn_sb[:, c * 1024:(c + 1) * 1024],
                          in_=ap["xn"][:, c * 1024:(c + 1) * 1024])

    # ---- phase A: A^T[e',q] = sum_e M[e,e'] xTq[e,q]
    # et-major with 8 concurrent PSUM groups (one bank per e'-tile): the PE
    # consumes (m, xtq) DMA chunk pairs in arrival order instead of waiting
    # for the full 4MB before the first accumulation group can close.
    with tc.tile_pool(name="pa", bufs=1, space="PSUM") as pa:
        for hf in range(2):
            ps = [pa.tile([P, 512], f32, name=f"pa{ep}", tag=f"pa{ep}")
                  for ep in range(8)]
            for et in range(8):
                for ep in range(8):
                    nc.tensor.matmul(
                        ps[ep],
                        m_sb[:, et * 1024 + ep * P: et * 1024 + (ep + 1) * P],
                        xtq_sb[:, et * 1024 + hf * 512: et * 1024 + (hf + 1) * 512],
                        start=(et == 0), stop=(et == 7))
                    if et == 7:
                        # evict right after each group's stop, half on each
                        # engine, so the last bank frees ASAP for phase B
                        base = ep * 1024 + hf * 512
                        nc.scalar.copy(at_sb[:, base: base + 256],
                                       ps[ep][:, 0:256])
                        nc.vector.tensor_copy(at_sb[:, base + 256: base + 512],
                                              ps[ep][:, 256:512])

    # ---- phase B pools (PSUM banks: sp 2 + zt 4 + rs 1 + op 1 = 8)
    sp = ctx.enter_context(tc.tile_pool(name="sp", bufs=1, space="PSUM"))
    ztp = ctx.enter_context(tc.tile_pool(name="ztp", bufs=2, space="PSUM"))
    rsp = ctx.enter_context(tc.tile_pool(name="rsp", bufs=1, space="PSUM"))
    opp = ctx.enter_context(tc.tile_pool(name="opp", bufs=1, space="PSUM"))
    ptp = ctx.enter_context(tc.tile_pool(name="ptp", bufs=PIPE + 3))
    ztsb = ctx.enter_context(tc.tile_pool(name="ztsb", bufs=2))
    rvp = ctx.enter_context(tc.tile_pool(name="rvp", bufs=2))
    osp = ctx.enter_context(tc.tile_pool(name="osp", bufs=3))

    out_t = ap["out"].rearrange("(t p) f -> t p f", p=P)

    steps = [(t, st) for t in range(NQT) for st in range(2 * t + 2)]
    rs = rsp.tile([P, 1], f32, name="rs")  # one bank, reused every t
    state = {}   # t -> zt tiles ([128,512] x2, 4 e-slices each)
    zts_of = {}  # t -> evicted SBUF zt tiles
    rinv_of = {}
    # Two score banks, alternating per step: a matmul group's start=True
    # write-locks its whole 2KB zero region, so consecutive steps must use
    # different banks or each step serializes on the previous step's exp read.
    spcur = {}   # parity -> rolling [128,512] tile, 4 st-slices

    def emit_scores(i, t, st):
        g = 2 * t + 1
        par, n = i % 2, i // 2
        if n % 4 == 0:
            spcur[par] = sp.tile([P, 512], f32, name=f"sps{par}",
                                 tag=f"sp{par}")
        ps = spcur[par][:, (n % 4) * P:(n % 4 + 1) * P]
        for ep in range(8):
            nc.tensor.matmul(
                ps,
                xts_sb[:, ep * 2048 + st * P: ep * 2048 + (st + 1) * P],
                at_sb[:, ep * 1024 + t * P: ep * 1024 + (t + 1) * P],
                start=(ep == 0), stop=(ep == 7))
        pt = ptp.tile([P, P], bf16, name="pt", tag="pt")
        nc.scalar.activation(pt, ps, Exp)
        if st == g - 1:
            nc.vector.tensor_mul(pt, pt, maskp)
        elif st == g:
            nc.vector.tensor_mul(pt, pt, maskl)
        return pt

    def emit_op(t, hf, alt_bank=False, strips=1):
        zs = zts_of[t]
        rv = rinv_of[t]
        if alt_bank:  # final OP: spare zt slot, avoids WAR on the op bank
            po = ztp.tile([P, 512], f32, name="po2", tag="zt0")
        else:
            po = opp.tile([P, 512], f32, name="po", tag="op")
        for ez in range(8):
            nc.tensor.matmul(
                po, zs[ez // 4][:, (ez % 4) * P:(ez % 4 + 1) * P],
                wvt_sb[:, ez * 1024 + hf * 512: ez * 1024 + (hf + 1) * 512],
                start=(ez == 0), stop=(ez == 7))
        w = 512 // strips
        for s in range(strips):
            ob = osp.tile([P, w], f32, name="ob", tag=f"ob{s}")
            nc.scalar.activation(ob, po[:, s * w:(s + 1) * w], Copy, scale=rv)
            nc.sync.dma_start(
                out=out_t[t][:, hf * 512 + s * w: hf * 512 + (s + 1) * w],
                in_=ob)

    def emit_rz(t, st, pt):
        g = 2 * t + 1
        if st == 0:
            state[t] = [ztp.tile([P, 512], f32, name=f"zt{j}", tag=f"zt{j}")
                        for j in range(2)]
        zt = state[t]
        nc.tensor.matmul(rs, pt, ones, start=(st == 0), stop=(st == g))
        for ez in range(8):
            # one accumulation group per zt tile: start/stop only on that
            # tile's first/last matmul of the whole st loop (2KB zero region)
            nc.tensor.matmul(
                zt[ez // 4][:, (ez % 4) * P:(ez % 4 + 1) * P],
                xn_sb[:, st * 1024 + ez * P: st * 1024 + (ez + 1) * P],
                pt,
                start=(st == 0 and ez % 4 == 0),
                stop=(st == g and ez % 4 == 3))
        if st == g:
            zs = []
            for j in range(2):
                z = ztsb.tile([P, 512], bf16, name=f"zs{j}", tag=f"zs{j}")
                if t == NQT - 1:  # tail: strip across both engines
                    nc.scalar.copy(z[:, 0:256], zt[j][:, 0:256])
                    nc.vector.tensor_copy(z[:, 256:512], zt[j][:, 256:512])
                elif j == 0:
                    nc.scalar.copy(z, zt[j])
                else:
                    nc.vector.tensor_copy(z, zt[j])
                zs.append(z)
            zts_of[t] = zs
            rv = rvp.tile([P, 1], f32, name="rv", tag="rv")
            nc.vector.reciprocal(rv, rs)
            rinv_of[t] = rv
            del state[t]
        elif t > 0 and st == 0:
            emit_op(t - 1, 0)
        elif t > 0 and st == 2:
            emit_op(t - 1, 1)

    pend = []
    for i in range(len(steps) + PIPE):
        if i < len(steps):
            t, st = steps[i]
            pend.append((t, st, emit_scores(i, t, st)))
        if i >= PIPE:
            t, st, pt = pend.pop(0)
            emit_rz(t, st, pt)
    emit_op(NQT - 1, 0, strips=2)
    emit_op(NQT - 1, 1, alt_bank=True, strips=2)


def build_program():
    if "nc" in _prog_cache:
        return _prog_cache["nc"]
    from contextlib import ExitStack
    from concourse import bacc, mybir
    import concourse.tile as tile

    nc = bacc.Bacc("TRN2", target_bir_lowering=False, debug=False,
                   num_devices=NCORES)
    f32 = mybir.dt.float32
    bf16 = mybir.dt.bfloat16
    ap = {
        "m": nc.dram_tensor("m", [P, 8 * 1024], bf16, kind="ExternalInput").ap(),
        "xtq": nc.dram_tensor("xtq", [P, 8 * 1024], bf16, kind="ExternalInput").ap(),
        "xts": nc.dram_tensor("xts", [P, 16 * 1024], bf16, kind="ExternalInput").ap(),
        "xn": nc.dram_tensor("xn", [P, 16 * 1024], bf16, kind="ExternalInput").ap(),
        "wvt": nc.dram_tensor("wvt", [P, 8 * 1024], bf16, kind="ExternalInput").ap(),
        "maskp": nc.dram_tensor("maskp", [P, P], bf16, kind="ExternalInput").ap(),
        "maskl": nc.dram_tensor("maskl", [P, P], bf16, kind="ExternalInput").ap(),
        "out": nc.dram_tensor("out", [1024, E], f32, kind="ExternalOutput").ap(),
    }
    with tile.TileContext(nc) as tc:
        with ExitStack() as ctx:
            _build_body(ctx, tc, ap)
    nc.compile()
    _prog_cache["nc"] = nc
    return nc


def _fold(a, nt, cols):
    # [nt*128, cols] -> [128, nt*cols] with block j at cols [j*cols:(j+1)*cols]
    return np.ascontiguousarray(
        a.reshape(nt, P, cols).transpose(1, 0, 2).reshape(P, nt * cols))


def make_in_maps(x, W_q, W_k, W_v):
    import ml_dtypes
    bf = ml_dtypes.bfloat16
    x = np.asarray(x, np.float32)
    W_q = np.asarray(W_q, np.float32)
    W_k = np.asarray(W_k, np.float32)
    W_v = np.asarray(W_v, np.float32)

    M = (W_q.T @ W_k) * SCALE                      # [e, e'], scale folded
    m_p = _fold(M, 8, 1024).astype(bf)
    wvt_p = _fold(np.ascontiguousarray(W_v.T), 8, 1024).astype(bf)

    i = np.arange(P)[:, None]
    j = np.arange(P)[None, :]
    tri = (i <= j).astype(np.float32)              # allow s_local <= q_local
    masks = [(np.ones((P, P), np.float32), tri),   # h=0: odd tiles, diag last
             (tri, np.zeros((P, P), np.float32))]  # h=1: even tiles

    in_maps = []
    for c in range(NCORES):
        b, h = c // 2, c % 2
        xb = x[b]                                  # [2048, 1024]
        xT = np.ascontiguousarray(xb.T)            # [1024, 2048]
        qcols = np.concatenate(
            [np.arange((2 * t + 1 - h) * P, (2 * t + 2 - h) * P)
             for t in range(NQT)])
        xq = np.ascontiguousarray(xb[qcols].T)     # [1024 e, 1024 q]
        mp, ml = masks[h]
        in_maps.append({
            "m": m_p,
            "xtq": _fold(xq, 8, 1024).astype(bf),
            "xts": _fold(xT, 8, 2048).astype(bf),
            "xn": _fold(xb, 16, 1024).astype(bf),
            "wvt": wvt_p,
            "maskp": mp.astype(bf),
            "maskl": ml.astype(bf),
        })
    return in_maps


def assemble(results):
    out = np.zeros((B, S, E), np.float32)
    for c in range(NCORES):
        b, h = c // 2, c % 2
        co = results[c]["out"]
        for t in range(NQT):
            g = 2 * t + (1 - h)
            out[b, g * P:(g + 1) * P, :] = co[t * P:(t + 1) * P]
    return out


def kernel(x, W_q, W_k, W_v):
    from concourse.bass_utils import run_bass_kernel_spmd
    nc = build_program()
    in_maps = make_in_maps(x, W_q, W_k, W_v)
    res = run_bass_kernel_spmd(nc, in_maps, core_ids=list(range(NCORES)))
    return assemble(res.results)
